# revision 12
# baseline (speedup 1.0000x reference)
"""ParallelRetention Trainium2 Bass kernel (v2: pipelined build).

Problem (per [b,h] slice, B=2 H=16 S=2048 D=64):
    decay  = omask / sqrt(rowsum(omask))          (per-row rsqrt scale)
    ret    = (q @ k^T) * decay
    denom  = clip(|rowsum(ret)|, 1, inf)
    out    = (ret / denom) @ v
Restructured (see kernel_baseline.py.bak for the derivation):
    augT   = [v | 1]^T @ (scores * omask)^T       # [65, S] per (b,h)
    rs[q]  = 1/sqrt(max(msum[q], tiny)),  msum = rowsum(omask)
    out[q] = augT[0:64, q] * rs[q] / max(|augT[64, q] * rs[q]|, 1)

v2 changes vs baseline:
  - The per-chunk omaskT build (PE transposes + ACT copies) and the msum
    row-sum pass are software-pipelined INTO the previous chunk's matmul
    stream instead of running as a separate phase.
  - omask DMA for chunk ch+1 issues at the top of stream(ch).
  - msum can run on DVE via fused tensor_scalar+accum_out instead of ACT
    activation accumulate (ACT was the most loaded engine).
  - A tunable number of score*omask multiplies go to GPSIMD (otherwise
    idle) via ACT PSUM->SBUF copies; the rest multiply on DVE directly
    from PSUM.

Sharding: 16 heads / 8 cores = 2 heads per core, both batches on the same
core (omask is per-head, halving omask traffic per core). SPMD: one NEFF,
per-core input slices.
"""

import os

import numpy as np

B = 2
H = 16
S = 2048
D = 64
N_CORES = 8
HC = H // N_CORES
P = 128
QT = S // P
KT = S // P
QC = 512
NCH = S // QC
TPC = QC // P

GP_QUADS = int(os.environ.get("KRN_GP_QUADS", "2"))   # 0..8 gpsimd quads/chunk
MSUM_DVE = int(os.environ.get("KRN_MSUM_DVE", "0"))   # 0..4 msum tiles on DVE
GP_DEFER = int(os.environ.get("KRN_GP_DEFER", "2"))   # jj slots to defer quad
                                                      # aug MMs (hide gpsimd)
AUG_DEFER = int(os.environ.get("KRN_AUG_DEFER", "1"))  # jj slots to defer the
                                                       # DVE-path aug MMs
SG8 = os.environ.get("KRN_SG8", "0") == "1"           # 2 k-blocks per stage
PIPE = os.environ.get("KRN_PIPE", "1") == "1"         # pipelined build
N_WARMUP = int(os.environ.get("KRN_WARMUP", "18"))
FILLERS = int(os.environ.get("KRN_FILLERS", "0"))

_NC_CACHE = {}


def _build_nc():
    import concourse.mybir as mybir
    import concourse.tile as tile
    from concourse import bacc
    from concourse.masks import make_identity

    F32R = mybir.dt.float32r
    F32 = mybir.dt.float32
    BF = mybir.dt.bfloat16
    MULT = mybir.AluOpType.mult
    ADD = mybir.AluOpType.add

    KPG = 2 if SG8 else 1          # k-blocks per transpose stage group
    NGRP = KT // KPG               # stage groups per chunk (8 or 16)
    PGSZ = 8 if SG8 else 4         # transposes per prep stage group

    # build-step placement across the 8 jj slots of the previous chunk's
    # stream: group counts per jj (sums to NGRP), msum tile index per jj
    if SG8:
        GRP_AT = [0, 0, 2, 1, 1, 1, 1, 2]
    else:
        GRP_AT = [0, 0, 3, 3, 3, 3, 2, 2]
    MSUM_AT = {3: 0, 5: 1, 6: 2, 7: 3}   # jj -> onat tile index

    nc = bacc.Bacc("TRN2", target_bir_lowering=False, debug=False,
                   num_devices=N_CORES)

    q_d = nc.dram_tensor("q", [B, HC, S, D], F32, kind="ExternalInput")
    k_d = nc.dram_tensor("k", [B, HC, S, D], F32, kind="ExternalInput")
    v_d = nc.dram_tensor("v", [B, HC, S, D], F32, kind="ExternalInput")
    om_d = nc.dram_tensor("omask", [HC, S, S], F32, kind="ExternalInput")
    out_d = nc.dram_tensor("out", [B, HC, S, D], F32, kind="ExternalOutput")

    with tile.TileContext(nc) as tc:
        with (
            tc.tile_pool(name="const", bufs=1) as const_pool,
            tc.tile_pool(name="onat", bufs=4) as onat_pool,
            tc.tile_pool(name="omt", bufs=3) as omt_pool,
            tc.tile_pool(name="qkv", bufs=2) as qkv_pool,
            tc.tile_pool(name="work", bufs=(4 if AUG_DEFER else 3)) \
                as work_pool,
            tc.tile_pool(name="quad", bufs=2) as quad_pool,
            tc.tile_pool(name="small", bufs=4) as small_pool,
            tc.tile_pool(name="outp", bufs=2) as out_pool,
            tc.tile_pool(name="mdum", bufs=1) as mdum_pool,
            tc.tile_pool(name="ps_sc", bufs=2, space="PSUM") as ps_sc,
            tc.tile_pool(name="ps_stage", bufs=(1 if SG8 else 2),
                         space="PSUM") as ps_stage,
            tc.tile_pool(name="ps_aug", bufs=2, space="PSUM") as ps_aug,
        ):
            ident_f = const_pool.tile([P, P], F32, tag="ident_f")
            make_identity(nc, ident_f)

            # PE warmup: back-to-back matmuls so the HAM clock gate lifts
            # (1.2 -> 2.4 GHz) before the real stream; overlaps first DMAs.
            warm_w = const_pool.tile([P, P], F32R, tag="warm_w")
            nc.vector.tensor_copy(warm_w, ident_f)
            warm_xf = const_pool.tile([P, QC], F32, tag="warm_xf")
            nc.vector.memset(warm_xf, 1.0)
            warm_x = const_pool.tile([P, QC], F32R, tag="warm_x")
            nc.vector.tensor_copy(warm_x, warm_xf)
            if N_WARMUP:
                warm_ps = ps_aug.tile([P, QC], F32, tag="aug")
                for _ in range(N_WARMUP):
                    nc.tensor.matmul(warm_ps, warm_w, warm_x,
                                     start=True, stop=True)
                warm_sink = small_pool.tile([P, 1], F32, tag="warm_sink")
                nc.vector.tensor_copy(warm_sink, warm_ps[:, 0:1])

            # msum dummy outputs (never read; accum_out carries the result)
            mdum_dve = None
            mdum_act = None
            if MSUM_DVE > 0:
                mdum_dve = mdum_pool.tile([P, S], BF, tag="mdum_dve")
            if MSUM_DVE < TPC:
                mdum_act = mdum_pool.tile([P, S], BF, tag="mdum_act")

            def prep_inputs(b, h):
                # q tiles with the d-column block duplicated ([p, t, 128] =
                # [q | q]) and k tiles packed pairwise; a [128,128] PE
                # transpose of each yields qT duplicated into both partition
                # halves and kT pairs split 0-63/64-127 so the two K=64
                # score matmuls of a pair run in disjoint PE row-groups.
                qsrc = q_d[b, h].rearrange("(t p) d -> p t d", p=P)
                qn2 = qkv_pool.tile([P, QT, P], F32, tag="qn")
                nc.sync.dma_start(out=qn2[:, :, 0:D], in_=qsrc)
                nc.sync.dma_start(out=qn2[:, :, D:2 * D], in_=qsrc)
                kn2 = qkv_pool.tile([P, KT // 2, 2, D], F32, tag="kn")
                nc.sync.dma_start(
                    out=kn2,
                    in_=k_d[b, h].rearrange(
                        "(jj two p) d -> p jj two d", p=P, two=2))
                vn = qkv_pool.tile([P, KT, D], F32, tag="vn")
                nc.sync.dma_start(
                    out=vn,
                    in_=v_d[b, h].rearrange("(t p) d -> p t d", p=P))
                va = qkv_pool.tile([P, KT, D + 1], F32R, tag="va")
                nc.vector.tensor_copy(va[:, :, 0:D], vn)
                onesf = small_pool.tile([P, KT], F32, tag="onesf")
                nc.vector.memset(onesf, 1.0)
                nc.vector.tensor_copy(
                    va[:, :, D:D + 1].rearrange("p t o -> p (t o)"), onesf)

                qT = qkv_pool.tile([P, S], F32R, tag="qT")
                for g in range(QT // PGSZ):
                    stg = ps_stage.tile([P, PGSZ, P], F32, tag="stage")
                    for i in range(PGSZ):
                        nc.tensor.transpose(
                            stg[:, i, :], qn2[:, g * PGSZ + i, :], ident_f)
                    nc.scalar.copy(
                        out=qT[:, g * PGSZ * P:(g + 1) * PGSZ * P]
                            .rearrange("d (i c) -> d i c", c=P),
                        in_=stg)
                kT = qkv_pool.tile([P, KT // 2, P], F32R, tag="kT")
                kg = min(PGSZ, KT // 2)
                for g in range((KT // 2) // kg):
                    stg = ps_stage.tile([P, kg, P], F32, tag="stage")
                    for i in range(kg):
                        nc.tensor.transpose(
                            stg[:, i, :],
                            kn2[:, g * kg + i, :, :]
                                .rearrange("p two d -> p (two d)"),
                            ident_f)
                    nc.scalar.copy(
                        out=kT[:, g * kg:(g + 1) * kg, :], in_=stg)
                return qT, kT, va

            # GPSIMD quad assignment: (b, quad) pairs, spread across jj & b
            gps_set = set(
                [(0, 1), (1, 2), (1, 0), (0, 2), (0, 0), (1, 1), (0, 3),
                 (1, 3)][:GP_QUADS])

            for h in range(HC):
                msum = small_pool.tile([P, QT], F32, tag="msum")
                rs = small_pool.tile([P, QT], F32, tag="rs")

                prepped = [prep_inputs(b, h) for b in range(B)]

                def issue_onat_dma(ch):
                    onats = []
                    for t in range(TPC):
                        onat = onat_pool.tile([P, S], F32, tag="onat")
                        r0 = ch * QC + t * P
                        nc.sync.dma_start(
                            out=onat, in_=om_d[h, r0:r0 + P, :])
                        onats.append(onat)
                    return onats

                def alloc_omts():
                    return [omt_pool.tile([P, KT // 2, QC], F32, tag="omt",
                                          name=f"omt{i}")
                            for i in range(2)]

                def make_group_steps(onats, omts):
                    """NGRP closures: each transposes KPG k-blocks (TPC
                    row-tiles each) through PSUM and ACT-copies them into
                    the omt halves."""
                    def make_step(g):
                        def step():
                            j0 = g * KPG
                            stg = ps_stage.tile([P, KPG, TPC, P], F32,
                                                tag="stage")
                            for i in range(KPG):
                                j = j0 + i
                                for t in range(TPC):
                                    nc.tensor.transpose(
                                        stg[:, i, t, :],
                                        onats[t][:, j * P:(j + 1) * P],
                                        ident_f)
                            half = j0 // (KT // 2)
                            j8 = j0 % (KT // 2)
                            nc.scalar.copy(
                                out=omts[half][:, j8:j8 + KPG, :]
                                    .rearrange("p i (t c) -> p i t c", c=P),
                                in_=stg)
                        return step
                    return [make_step(g) for g in range(NGRP)]

                def msum_op(onats, ch, t):
                    qt = ch * TPC + t
                    if t < MSUM_DVE:
                        nc.vector.tensor_scalar(
                            mdum_dve, onats[t], 1.0, 0.0, MULT, ADD,
                            accum_out=msum[:, qt:qt + 1])
                    else:
                        nc.scalar.activation(
                            mdum_act, onats[t],
                            mybir.ActivationFunctionType.Copy,
                            accum_out=msum[:, qt:qt + 1])

                def rs_ops(ch):
                    csl = slice(ch * TPC, (ch + 1) * TPC)
                    nc.vector.tensor_scalar_max(
                        msum[:, csl], msum[:, csl], 1e-30)
                    nc.scalar.sqrt(rs[:, csl], msum[:, csl])
                    nc.vector.reciprocal(rs[:, csl], rs[:, csl])

                def full_build(ch):
                    onats = issue_onat_dma(ch)
                    omts = alloc_omts()
                    for step in make_group_steps(onats, omts):
                        step()
                    for t in range(TPC):
                        msum_op(onats, ch, t)
                    rs_ops(ch)
                    return omts

                # chunk 0 of each head: built as its own phase (overlaps
                # warmup / previous head's stream tail via Tile scheduling)
                omts = full_build(0)

                for ch in range(NCH):
                    csl = slice(ch * TPC, (ch + 1) * TPC)
                    gsteps, onats_n, omts_n = [], None, None
                    if ch + 1 < NCH:
                        onats_n = issue_onat_dma(ch + 1)
                        omts_n = alloc_omts()
                        gsteps = make_group_steps(onats_n, omts_n)
                        if not PIPE:
                            for step in gsteps:
                                step()
                            gsteps = []
                            for t in range(TPC):
                                msum_op(onats_n, ch + 1, t)
                            rs_ops(ch + 1)

                    def omt_at(jj):
                        half = (jj * 2) // (KT // 2)
                        j8 = (jj * 2) % (KT // 2)
                        return omts[half][:, j8:j8 + 2, :]

                    aug_ps = {}
                    pend = {}
                    started = {}
                    issued = {b: 0 for b in range(B)}
                    deferred = []   # (flush_jj, b, retq, jj0)
                    for b in range(B):
                        aug_b = ps_aug.tile([D + 1, QC], F32, tag="aug")
                        aug_ps[b] = aug_b
                        pend[b] = None
                        started[b] = False

                    def mm2(b, j, src_ap):
                        va = prepped[b][2]
                        issued[b] += 1
                        nc.tensor.matmul(
                            aug_ps[b], va[:, j, :], src_ap,
                            start=not started[b],
                            stop=(issued[b] == KT),
                            skip_group_check=True)
                        started[b] = True

                    def flush_deferred(jj_now):
                        for item in list(deferred):
                            fjj, b, mms = item
                            if jj_now < fjj:
                                continue
                            for j, ap in mms:
                                mm2(b, j, ap)
                            deferred.remove(item)

                    gi = 0
                    for jj in range(KT // 2):
                        for b in range(B):
                            qT, kT, va = prepped[b]
                            sc = ps_sc.tile([P, 2, QC], F32, tag="scores")
                            for j2 in range(2):
                                base = j2 * D
                                nc.tensor.matmul(
                                    sc[:, j2, :], kT[base:base + D, jj, :],
                                    qT[base:base + D,
                                       ch * QC:(ch + 1) * QC],
                                    start=True, stop=True)
                            if (b, jj // 2) in gps_set:
                                if pend[b] is None:
                                    scq = quad_pool.tile([P, 4, QC], F32,
                                                         tag="scq")
                                    retq = quad_pool.tile([P, 4, QC], F32R,
                                                          tag="retq")
                                    pend[b] = (scq, retq, jj)
                                scq, retq, jj0 = pend[b]
                                off = (jj - jj0) * 2
                                nc.scalar.copy(
                                    out=scq[:, off:off + 2, :], in_=sc)
                                if off == 2:
                                    half = (jj0 * 2) // (KT // 2)
                                    j8 = (jj0 * 2) % (KT // 2)
                                    nc.gpsimd.tensor_mul(
                                        retq, scq,
                                        omts[half][:, j8:j8 + 4, :])
                                    deferred.append((
                                        jj + GP_DEFER, b,
                                        [(jj0 * 2 + jq, retq[:, jq, :])
                                         for jq in range(4)]))
                                    pend[b] = None
                            else:
                                ret = work_pool.tile([P, 2, QC], F32R,
                                                     tag="ret")
                                nc.vector.tensor_mul(ret, sc, omt_at(jj))
                                mms = [(jj * 2 + j2, ret[:, j2, :])
                                       for j2 in range(2)]
                                if AUG_DEFER:
                                    deferred.append(
                                        (jj + AUG_DEFER, b, mms))
                                else:
                                    for j, ap in mms:
                                        mm2(b, j, ap)
                            flush_deferred(jj)
                        if PIPE and gsteps is not None:
                            for _ in range(GRP_AT[jj]):
                                if gi < len(gsteps):
                                    gsteps[gi]()
                                    gi += 1
                            if onats_n is not None and jj in MSUM_AT:
                                msum_op(onats_n, ch + 1, MSUM_AT[jj])
                        if FILLERS:
                            fill = ps_stage.tile([P, D], F32, tag="stage")
                            for _ in range(FILLERS):
                                nc.tensor.matmul(
                                    fill, warm_w, warm_x[:, 0:D],
                                    start=True, stop=True)
                    flush_deferred(10 ** 9)
                    if PIPE and ch + 1 < NCH:
                        rs_ops(ch + 1)

                    for b in range(B):
                        # postprocess: [65, QC] -> scaled [q, d] output
                        augs = out_pool.tile([D + 1, QC], F32, tag="augs")
                        nc.scalar.copy(out=augs, in_=aug_ps[b])
                        autp = ps_stage.tile([P, TPC, D + 1], F32,
                                             tag="stage")
                        for t in range(TPC):
                            nc.tensor.transpose(
                                autp[:, t, :], augs[:, t * P:(t + 1) * P],
                                ident_f[0:D + 1, 0:D + 1])
                        scal = small_pool.tile([P, TPC], F32, tag="scal")
                        nc.vector.tensor_mul(
                            scal,
                            autp[:, :, D:D + 1].rearrange(
                                "p t o -> p (t o)"),
                            rs[:, csl])
                        nc.scalar.activation(
                            scal, scal, mybir.ActivationFunctionType.Abs)
                        nc.vector.tensor_scalar_max(scal, scal, 1.0)
                        nc.vector.reciprocal(scal, scal)
                        nc.vector.tensor_mul(scal, scal, rs[:, csl])
                        ob = out_pool.tile([P, TPC, D], F32, tag="ob")
                        for t in range(TPC):
                            # per-partition scale is native on ScalarE
                            nc.scalar.activation(
                                ob[:, t, :], autp[:, t, 0:D],
                                mybir.ActivationFunctionType.Copy,
                                scale=scal[:, t:t + 1])
                        nc.sync.dma_start(
                            out=out_d[b, h, ch * QC:(ch + 1) * QC, :]
                                .rearrange("(t p) d -> p t d", p=P),
                            in_=ob)

                    omts = omts_n

    nc.compile()
    return nc


def _get_nc():
    if "nc" not in _NC_CACHE:
        _NC_CACHE["nc"] = _build_nc()
    return _NC_CACHE["nc"]


def kernel(q, k, v, omask, _trace=False):
    from concourse.bass_utils import run_bass_kernel_spmd

    nc = _get_nc()
    in_maps = []
    for c in range(N_CORES):
        hs = slice(c * HC, (c + 1) * HC)
        in_maps.append({
            "q": np.ascontiguousarray(q[:, hs]),
            "k": np.ascontiguousarray(k[:, hs]),
            "v": np.ascontiguousarray(v[:, hs]),
            "omask": np.ascontiguousarray(omask[hs]),
        })
    res = run_bass_kernel_spmd(nc, in_maps, core_ids=list(range(N_CORES)),
                               trace=_trace)
    out = np.concatenate([res.results[c]["out"] for c in range(N_CORES)],
                         axis=1)
    if _trace:
        kernel.last_results = res
    return out
